# revision 12
# baseline (speedup 1.0000x reference)
"""AdaptivePoolAttention Trainium2 kernel (8 NeuronCores, SPMD). v2

Structure (per core: batch b=c//2, token half c%2 -> 32 of 64 tokens):
 - AdaptiveAvgPool commutes with the qkv linear, so pool x first: the device
   streams x (bf16) and reduces 196 spatial rows -> 1 per token.
 - Spatial pre-add in the DMA engine: host splits each token's 196 rows into
   two halves of 98; half A is DMA'd plain, half B with accum_op=add into the
   same SBUF tiles. Halves the selector-matmul work (25 tiles instead of 49)
   at zero engine cost.
 - Selector matmul (bf16, TensorE) finishes the 98-row group means.
 - Weights/params stream strictly AFTER x (they'd otherwise delay the pooled
   result); ordered by tail need: wq first, wkv next, wp last.
 - Pairwise AllGather (cores 2c, 2c+1) of pooled halves -> full (64, 768).
 - Tail engine split: q path (proj+LN+rel-pos bias) on gpsimd/PE overlapping
   the collective; kv path (proj+LN) on vector/PE; exp/sqrt on ACT with table
   loads sequenced sqrt,sqrt,exp to avoid thrash; bf16 intermediates halve
   DVE cost. The rel-pos bias bias[t,s,h] is one GEMM G = q @ R^T over 127
   distances plus an affine-strided gather DMA through DRAM.
"""

import numpy as np
from contextlib import ExitStack

B, T, NH, HD, D = 4, 64, 12, 64, 768
S = 196            # 14*14 spatial positions
SHALF = 98         # spatial half pre-added by DMA
TLOC = 32          # tokens per core
RHALF = TLOC * SHALF   # 3136 valid rows per half
NT = 25            # 128-row tiles per half (3200 rows padded)
RPAD = NT * 128    # 3200
KB = D // 128      # 6 contraction tiles of 128
NDIST = 2 * T - 1  # 127 distinct temporal distances
SCALE = HD ** -0.5
LN_EPS = 1e-5
N_CORES = 8
GROUPS = (1, 2, 4, 6, 6, 3, 2, 1)  # progressive tile groups, sum = NT
USE_DMA_ACCUM = True

_BUILD_CACHE = {}


def _build_nc():
    import concourse.bass as bass
    import concourse.bacc as bacc
    import concourse.tile as tile
    import concourse.mybir as mybir
    from concourse.tile_rust import add_dep_helper

    f32 = mybir.dt.float32
    bf16 = mybir.dt.bfloat16
    AX = mybir.AxisListType.X
    OP = mybir.AluOpType

    nc = bacc.Bacc(
        "TRN2", target_bir_lowering=False, debug=False, num_devices=N_CORES,
    )

    xa = nc.declare_dram_parameter("xa", [RPAD, D], bf16, isOutput=False)
    xb = nc.declare_dram_parameter("xb", [RPAD, D], bf16, isOutput=False)
    sel = nc.declare_dram_parameter("sel", [128, NT, TLOC], bf16, isOutput=False)
    idp = nc.declare_dram_parameter("idp", [128, 128], bf16, isOutput=False)
    wq = nc.declare_dram_parameter("wq", [D, D], bf16, isOutput=False)
    wkv = nc.declare_dram_parameter("wkv", [D, 2 * D], bf16, isOutput=False)
    wp = nc.declare_dram_parameter("wp", [D, D], bf16, isOutput=False)
    rptt = nc.declare_dram_parameter("rptt", [HD, 128], bf16, isOutput=False)
    # prow planes: 0=gq 1=bq 2=bkrow 3=bvrow 4=gvrow (bf16, row-broadcast)
    prow = nc.declare_dram_parameter("prow", [TLOC, 5, D], bf16, isOutput=False)
    bproj = nc.declare_dram_parameter("bproj", [TLOC, D], f32, isOutput=False)
    gkcol = nc.declare_dram_parameter("gkcol", [HD, 1], f32, isOutput=False)
    out_ext = nc.declare_dram_parameter("out", [TLOC, D], f32, isOutput=True)

    with ExitStack() as ctx:
        tc = ctx.enter_context(tile.TileContext(nc))
        const = ctx.enter_context(tc.tile_pool(name="const", bufs=1))
        xp = ctx.enter_context(tc.tile_pool(name="xp", bufs=1))
        sb = ctx.enter_context(tc.tile_pool(name="sb", bufs=1))
        pg = ctx.enter_context(tc.tile_pool(name="pg", bufs=2, space="PSUM"))
        pt = ctx.enter_context(tc.tile_pool(name="pt", bufs=2, space="PSUM"))
        dram = ctx.enter_context(tc.tile_pool(name="dram", bufs=1, space="DRAM"))

        eps_sb = const.tile([128, 1], f32, tag="eps")
        nc.vector.memset(eps_sb, LN_EPS)
        zero_sb = const.tile([128, 1], f32, tag="zero")
        nc.vector.memset(zero_sb, 0.0)

        # head-of-line DMAs: sel (sync ring), identity (scalar ring)
        sel_sb = const.tile([128, NT, TLOC], bf16, tag="sel")
        nc.sync.dma_start(out=sel_sb, in_=sel.ap())
        ident = const.tile([128, 128], bf16, tag="ident")
        nc.scalar.dma_start(out=ident, in_=idp.ap())

        # preload the ACT Sqrt table (both LNs use it before the exp switch)
        warm = sb.tile([1, 1], f32, tag="warm")
        nc.scalar.activation(
            out=warm, in_=zero_sb[0:1, :],
            func=mybir.ActivationFunctionType.Sqrt,
            bias=eps_sb[0:1, :], scale=1.0,
        )

        # tiny dummy AllGather wakes ncfw so the real collective is fast
        agw_in = dram.tile([1, 32], bf16, tag="agwi")
        agw_out = dram.tile([2, 32], bf16, tag="agwo")
        nc.gpsimd.dma_start(out=agw_in, in_=sel_sb[0:1, 0, :])
        nc.gpsimd.collective_compute(
            "AllGather",
            mybir.AluOpType.bypass,
            replica_groups=[[0, 1], [2, 3], [4, 5], [6, 7]],
            ins=[agw_in.opt()],
            outs=[agw_out.opt()],
        )

        # ---- phase 1: stream x halves; DMA pre-adds half B into half A ----
        xa_t = xa.ap().rearrange("(n p) d -> n p d", p=128)
        xb_t = xb.ap().rearrange("(n p) d -> n p d", p=128)
        m_psum = pg.tile([TLOC, D], f32, tag="g")
        xa_dmas, xb_dmas = [], []
        t_base = 0
        for g, gsz in enumerate(GROUPS):
            xt = xp.tile([128, gsz, D], bf16, tag=f"x{g}")
            src_a = bass.AP(
                tensor=xa_t.tensor,
                offset=xa_t.offset + t_base * 128 * D,
                ap=[[D, 128], [128 * D, gsz], [1, D]],
            )
            src_b = bass.AP(
                tensor=xb_t.tensor,
                offset=xb_t.offset + t_base * 128 * D,
                ap=[[D, 128], [128 * D, gsz], [1, D]],
            )
            ring_a = nc.sync if g % 2 == 0 else nc.scalar
            da = ring_a.dma_start(out=xt[:, 0:gsz, :], in_=src_a)
            if USE_DMA_ACCUM:
                # accum DMA is software-DGE (gpsimd) only
                db = nc.gpsimd.dma_start(
                    out=xt[:, 0:gsz, :], in_=src_b,
                    accum_op=mybir.AluOpType.add,
                )
                add_dep_helper(db.ins, da.ins, sync=True, reason="accum after base")
                xs_g = xt
            else:
                xbt = xp.tile([128, gsz, D], bf16, tag=f"xb{g}")
                ring_b = nc.scalar if g % 2 == 0 else nc.sync
                db = ring_b.dma_start(out=xbt[:, 0:gsz, :], in_=src_b)
                xs_g = xp.tile([128, gsz, D], bf16, tag=f"xs{g}")
                eng = nc.vector if g % 2 == 0 else nc.gpsimd
                eng.tensor_tensor(
                    out=xs_g[:, 0:gsz, :], in0=xt[:, 0:gsz, :],
                    in1=xbt[:, 0:gsz, :], op=OP.add,
                )
            xa_dmas.append(da)
            xb_dmas.append(db)
            for i in range(gsz):
                ti = t_base + i
                for c0, cw in ((0, 512), (512, 256)):
                    nc.tensor.matmul(
                        m_psum[:, c0:c0 + cw],
                        sel_sb[:, ti, :],
                        xs_g[:, i, c0:c0 + cw],
                        start=(ti == 0),
                        stop=(ti == NT - 1),
                    )
            t_base += gsz

        # ---- weights/params strictly after BOTH rings' x streams,
        # ordered by tail need: wq | small params, wkv | wp last ----
        def after_x(wd):
            add_dep_helper(wd.ins, xa_dmas[-1].ins, sync=False, reason="x before w")
            add_dep_helper(wd.ins, xb_dmas[-1].ins, sync=False, reason="x before w")

        wq_sb = const.tile([128, KB, D], bf16, tag="wq")
        after_x(nc.sync.dma_start(out=wq_sb, in_=wq.ap().rearrange("(k p) e -> p k e", p=128)))
        prow_sb = const.tile([TLOC, 5, D], bf16, tag="prow")
        after_x(nc.scalar.dma_start(out=prow_sb, in_=prow.ap()))
        rptt_sb = const.tile([HD, 128], bf16, tag="rptt")
        after_x(nc.scalar.dma_start(out=rptt_sb, in_=rptt.ap()))
        gkcol_sb = const.tile([HD, 1], f32, tag="gkcol")
        after_x(nc.scalar.dma_start(out=gkcol_sb, in_=gkcol.ap()))
        bproj_sb = const.tile([TLOC, D], f32, tag="bproj")
        after_x(nc.scalar.dma_start(out=bproj_sb, in_=bproj.ap()))
        wkv_sb = const.tile([128, KB, 2 * D], bf16, tag="wkv")
        after_x(nc.scalar.dma_start(out=wkv_sb, in_=wkv.ap().rearrange("(k p) e -> p k e", p=128)))
        wp_sb = const.tile([128, KB, D], bf16, tag="wp")
        after_x(nc.sync.dma_start(out=wp_sb, in_=wp.ap().rearrange("(k p) e -> p k e", p=128)))

        def bcast_free(ap2d, inner):
            # (P, F) AP -> (P, F, inner) AP with stride-0 innermost broadcast
            return bass.AP(
                tensor=ap2d.tensor,
                offset=ap2d.offset,
                ap=[*ap2d.ap, [0, inner]],
            )

        def plane(i):
            return prow_sb[:, i, :]

        def plane3(i):
            return prow_sb[:, i, :].rearrange("p (g d) -> p g d", g=NH)

        # ================= tail =================
        # T0: pooled m (32, 768) complete in PSUM
        m_sb = sb.tile([TLOC, D], bf16, tag="m")
        nc.vector.tensor_copy(out=m_sb, in_=m_psum)

        # collective path (gpsimd ring): m -> DRAM -> AllGather -> mf
        ag_in = dram.tile([TLOC, D], bf16, tag="agi")
        ag_out = dram.tile([T, D], bf16, tag="ago")
        nc.gpsimd.dma_start(out=ag_in, in_=m_sb)
        nc.gpsimd.collective_compute(
            "AllGather",
            mybir.AluOpType.bypass,
            replica_groups=[[0, 1], [2, 3], [4, 5], [6, 7]],
            ins=[ag_in.opt()],
            outs=[ag_out.opt()],
        )
        mf_sb = sb.tile([T, D], bf16, tag="mf")
        nc.gpsimd.dma_start(out=mf_sb, in_=ag_out)

        # local m^T then q projection
        mT_psum = pt.tile([128, KB, TLOC], bf16, tag="t")
        for k in range(KB):
            nc.tensor.matmul(
                mT_psum[:, k, :], m_sb[:, k * 128:(k + 1) * 128],
                ident[:TLOC, :TLOC], is_transpose=True,
            )
        mT_sb = sb.tile([128, KB, TLOC], bf16, tag="mT")
        nc.vector.tensor_copy(out=mT_sb, in_=mT_psum)

        q_psum = pg.tile([TLOC, D], f32, tag="g")
        for k in range(KB):
            for c0, cw in ((0, 512), (512, 256)):
                nc.tensor.matmul(
                    q_psum[:, c0:c0 + cw],
                    mT_sb[:, k, :],
                    wq_sb[:, k, c0:c0 + cw],
                    start=(k == 0), stop=(k == KB - 1),
                )

        # gathered tokens^T (PE waits on the collective here)
        mfT_psum = pt.tile([128, KB, T], bf16, tag="t")
        for k in range(KB):
            nc.tensor.matmul(
                mfT_psum[:, k, :], mf_sb[:, k * 128:(k + 1) * 128],
                ident[:T, :T], is_transpose=True,
            )
        mfT_sb = sb.tile([128, KB, T], bf16, tag="mfT")
        nc.vector.tensor_copy(out=mfT_sb, in_=mfT_psum)

        # kv projection: k rows 0:T, v rows T:128 of one (128, 768) PSUM tile
        kv_psum = pg.tile([128, D], f32, tag="g")
        for k in range(KB):
            for c0, cw in ((0, 512), (512, 256)):
                nc.tensor.matmul(
                    kv_psum[0:T, c0:c0 + cw],
                    mfT_sb[:, k, :],
                    wkv_sb[:, k, c0:c0 + cw],
                    start=(k == 0), stop=(k == KB - 1),
                )
            for c0, cw in ((0, 512), (512, 256)):
                nc.tensor.matmul(
                    kv_psum[T:128, c0:c0 + cw],
                    mfT_sb[:, k, :],
                    wkv_sb[:, k, D + c0:D + c0 + cw],
                    start=(k == 0), stop=(k == KB - 1),
                )

        # ---- q LayerNorm on gpsimd (sqrt on ACT, recip on vector) ----
        q3 = q_psum.rearrange("p (g d) -> p g d", g=NH)
        q_red = sb.tile([TLOC, NH], f32, tag="q_red")
        nc.vector.reduce_sum(out=q_red, in_=q3, axis=AX)
        q_mean = sb.tile([TLOC, NH], f32, tag="q_mean")
        nc.gpsimd.tensor_scalar_mul(out=q_mean, in0=q_red, scalar1=1.0 / HD)
        q_xc = sb.tile([TLOC, NH, HD], bf16, tag="q_xc")
        nc.vector.tensor_tensor(
            out=q_xc, in0=q3, in1=bcast_free(q_mean[:], HD), op=OP.subtract,
        )
        q_sq = sb.tile([TLOC, NH, HD], bf16, tag="q_sq")
        nc.gpsimd.tensor_mul(out=q_sq, in0=q_xc, in1=q_xc)
        q_var = sb.tile([TLOC, NH], f32, tag="q_var")
        nc.vector.reduce_sum(out=q_var, in_=q_sq, axis=AX)
        nc.scalar.activation(
            out=q_var, in_=q_var, func=mybir.ActivationFunctionType.Sqrt,
            bias=eps_sb[:TLOC], scale=1.0 / HD,
        )
        q_rstd = sb.tile([TLOC, NH], bf16, tag="q_rstd")
        with nc.allow_low_precision(reason="bf16 rstd: 0.4% on a norm scale"):
            nc.vector.reciprocal(out=q_rstd, in_=q_var)
        q_xcr = sb.tile([TLOC, NH, HD], bf16, tag="q_xcr")
        nc.gpsimd.tensor_tensor(
            out=q_xcr, in0=q_xc, in1=bcast_free(q_rstd[:], HD), op=OP.mult,
        )
        q_g = sb.tile([TLOC, NH, HD], bf16, tag="q_g")
        nc.gpsimd.tensor_mul(out=q_g, in0=q_xcr, in1=plane3(0))
        ln_q = sb.tile([TLOC, D], bf16, tag="lnq")
        nc.gpsimd.tensor_add(
            out=ln_q.rearrange("p (g d) -> p g d", g=NH), in0=q_g, in1=plane3(1),
        )
        # Bq[t,h] = sum_d ln_q * b_k  (k-LN beta folded into the bias)
        qbk = sb.tile([TLOC, NH, HD], bf16, tag="qbk")
        nc.gpsimd.tensor_mul(
            out=qbk, in0=ln_q.rearrange("p (g d) -> p g d", g=NH), in1=plane3(2),
        )
        bq_fold = sb.tile([TLOC, NH], bf16, tag="bqf")
        with nc.allow_low_precision(reason="bf16 folded beta_k bias term"):
            nc.vector.reduce_sum(out=bq_fold, in_=qbk, axis=AX)
        # residual + beta_v tile
        lnq_bv = sb.tile([TLOC, D], bf16, tag="lnqbv")
        nc.gpsimd.tensor_add(out=lnq_bv, in0=ln_q, in1=plane(3))

        # q^T per head
        qbT_psum = pt.tile([HD, NH, TLOC], bf16, tag="t")
        for h in range(NH):
            nc.tensor.matmul(
                qbT_psum[:, h, :], ln_q[:, h * HD:(h + 1) * HD],
                ident[:TLOC, :TLOC], is_transpose=True,
            )
        qbT_sb = sb.tile([HD, NH, TLOC], bf16, tag="qbT")
        nc.vector.tensor_copy(out=qbT_sb, in_=qbT_psum)

        # rel-pos bias: G = q @ Rflip^T then affine gather through DRAM
        g_psum = pg.tile([TLOC, NH, 128], f32, tag="g")
        for h in range(NH):
            nc.tensor.matmul(
                g_psum[:, h, :], qbT_sb[:, h, :], rptt_sb,
                start=True, stop=True,
            )
        g_sb = sb.tile([TLOC, NH, 128], bf16, tag="gsb")
        nc.scalar.activation(
            out=g_sb.rearrange("p h j -> p (h j)"),
            in_=g_psum.rearrange("p h j -> p (h j)"),
            func=mybir.ActivationFunctionType.Copy,
            bias=0.0, scale=1.0,
        )
        g_dram = dram.tile([TLOC, NH, 128], bf16, tag="gd")
        nc.gpsimd.dma_start(out=g_dram, in_=g_sb)
        bias_raw = sb.tile([TLOC, NH, T], bf16, tag="braw")
        gather_ap = bass.AP(
            tensor=g_dram.tensor,
            offset=g_dram.offset + 63,
            ap=[[NH * 128 - 1, TLOC], [128, NH], [1, T]],
        )
        nc.gpsimd.dma_start(out=bias_raw, in_=gather_ap)
        bias_sb = sb.tile([TLOC, NH, T], f32, tag="bias")
        nc.gpsimd.tensor_tensor(
            out=bias_sb, in0=bias_raw, in1=bcast_free(bq_fold[:], T), op=OP.add,
        )

        # ---- kv LayerNorm (normalized only) on vector ----
        kv3 = kv_psum.rearrange("p (g d) -> p g d", g=NH)
        kv_red = sb.tile([128, NH], f32, tag="kv_red")
        nc.vector.reduce_sum(out=kv_red, in_=kv3, axis=AX)
        kv_mean = sb.tile([128, NH], f32, tag="kv_mean")
        nc.vector.tensor_scalar_mul(out=kv_mean, in0=kv_red, scalar1=1.0 / HD)
        kv_xc = sb.tile([128, NH, HD], bf16, tag="kv_xc")
        nc.vector.tensor_tensor(
            out=kv_xc, in0=kv3, in1=bcast_free(kv_mean[:], HD), op=OP.subtract,
        )
        kv_sq = sb.tile([128, NH, HD], bf16, tag="kv_sq")
        nc.vector.tensor_mul(out=kv_sq, in0=kv_xc, in1=kv_xc)
        kv_var = sb.tile([128, NH], f32, tag="kv_var")
        nc.vector.reduce_sum(out=kv_var, in_=kv_sq, axis=AX)
        nc.scalar.activation(
            out=kv_var, in_=kv_var, func=mybir.ActivationFunctionType.Sqrt,
            bias=eps_sb, scale=1.0 / HD,
        )
        # trigger the exp table load now (softmax comes right after)
        expwarm = sb.tile([1, 1], f32, tag="expwarm")
        nc.scalar.activation(
            out=expwarm, in_=zero_sb[0:1, :],
            func=mybir.ActivationFunctionType.Exp,
            bias=zero_sb[0:1, :], scale=1.0,
        )
        kv_rstd = sb.tile([128, NH], bf16, tag="kv_rstd")
        with nc.allow_low_precision(reason="bf16 rstd: 0.4% on a norm scale"):
            nc.vector.reciprocal(out=kv_rstd, in_=kv_var)
        ln_kv = sb.tile([128, D], bf16, tag="lnkv")
        nc.vector.tensor_tensor(
            out=ln_kv.rearrange("p (g d) -> p g d", g=NH),
            in0=kv_xc, in1=bcast_free(kv_rstd[:], HD), op=OP.mult,
        )
        ln_v = sb.tile([T, D], bf16, tag="lnv")
        nc.vector.tensor_copy(out=ln_v, in_=ln_kv[T:128, :])

        # k^T per head with gamma_k applied per-partition
        kT_psum = pt.tile([HD, NH, T], bf16, tag="t")
        for h in range(NH):
            nc.tensor.matmul(
                kT_psum[:, h, :], ln_kv[0:T, h * HD:(h + 1) * HD],
                ident[:T, :T], is_transpose=True,
            )
        kT_sb = sb.tile([HD, NH, T], bf16, tag="kT")
        nc.vector.tensor_scalar_mul(
            out=kT_sb.rearrange("p h s -> p (h s)"),
            in0=kT_psum.rearrange("p h s -> p (h s)"),
            scalar1=gkcol_sb,
        )

        # scores, bias add, exp (scale folded into the activation)
        s_psum = pg.tile([TLOC, NH, T], f32, tag="g")
        for h in range(NH):
            nc.tensor.matmul(
                s_psum[:, h, :], qbT_sb[:, h, :], kT_sb[:, h, :],
                start=True, stop=True,
            )
        s_sb = sb.tile([TLOC, NH, T], bf16, tag="ssb")
        nc.vector.tensor_tensor(out=s_sb, in0=s_psum, in1=bias_sb, op=OP.add)
        p_sb = sb.tile([TLOC, NH, T], bf16, tag="p")
        nc.scalar.activation(
            out=p_sb.rearrange("p h s -> p (h s)"),
            in_=s_sb.rearrange("p h s -> p (h s)"),
            func=mybir.ActivationFunctionType.Exp,
            bias=zero_sb[:TLOC], scale=SCALE,
        )
        rsum = sb.tile([TLOC, NH], f32, tag="rsum")
        nc.vector.reduce_sum(out=rsum, in_=p_sb, axis=AX)
        rinv = sb.tile([TLOC, NH], f32, tag="rinv")
        nc.vector.reciprocal(out=rinv, in_=rsum)

        # P^T per head
        pT_psum = pt.tile([T, NH, TLOC], bf16, tag="t")
        for h in range(NH):
            nc.tensor.matmul(
                pT_psum[:, h, :], p_sb[:, h, :],
                ident[:TLOC, :TLOC], is_transpose=True,
            )
        pT_sb = sb.tile([T, NH, TLOC], bf16, tag="pT")
        nc.vector.tensor_copy(out=pT_sb, in_=pT_psum)

        # A@V per head
        o_psum = pg.tile([TLOC, NH, HD], f32, tag="g")
        for h in range(NH):
            nc.tensor.matmul(
                o_psum[:, h, :], pT_sb[:, h, :],
                ln_v[:, h * HD:(h + 1) * HD],
                start=True, stop=True,
            )
        # o = (P@v_hat)*(1/sum)*gamma_v + (ln_q + beta_v)
        o1 = sb.tile([TLOC, NH, HD], bf16, tag="o1")
        nc.vector.tensor_tensor(
            out=o1, in0=o_psum, in1=bcast_free(rinv[:], HD), op=OP.mult,
        )
        o2 = sb.tile([TLOC, NH, HD], bf16, tag="o2")
        nc.gpsimd.tensor_mul(out=o2, in0=o1, in1=plane3(4))
        o_sb = sb.tile([TLOC, D], bf16, tag="o")
        nc.gpsimd.tensor_add(
            out=o_sb, in0=o2.rearrange("p h d -> p (h d)"), in1=lnq_bv,
        )

        # o^T then output projection
        oT_psum = pt.tile([128, KB, TLOC], bf16, tag="t")
        for k in range(KB):
            nc.tensor.matmul(
                oT_psum[:, k, :], o_sb[:, k * 128:(k + 1) * 128],
                ident[:TLOC, :TLOC], is_transpose=True,
            )
        oT_sb = sb.tile([128, KB, TLOC], bf16, tag="oT")
        nc.vector.tensor_copy(out=oT_sb, in_=oT_psum)

        proj_psum = pg.tile([TLOC, D], f32, tag="g")
        out_sb = sb.tile([TLOC, D], f32, tag="outsb")
        for c0, cw in ((0, 512), (512, 256)):
            for k in range(KB):
                nc.tensor.matmul(
                    proj_psum[:, c0:c0 + cw],
                    oT_sb[:, k, :],
                    wp_sb[:, k, c0:c0 + cw],
                    start=(k == 0), stop=(k == KB - 1),
                )
            nc.vector.tensor_add(
                out=out_sb[:, c0:c0 + cw], in0=proj_psum[:, c0:c0 + cw],
                in1=bproj_sb[:, c0:c0 + cw],
            )
            nc.sync.dma_start(
                out=out_ext.ap()[:, c0:c0 + cw], in_=out_sb[:, c0:c0 + cw],
            )

    nc.compile()
    return nc


def _host_prep(x, W_qkv, g_q, b_q, g_k, b_k, g_v, b_v, W_proj, b_proj, rel_pos_t):
    import ml_dtypes
    bf = ml_dtypes.bfloat16
    x = np.asarray(x, np.float32)
    W_qkv = np.asarray(W_qkv, np.float32)
    W_proj = np.asarray(W_proj, np.float32)
    rel_pos_t = np.asarray(rel_pos_t, np.float32)

    # selector: 98 pre-added rows per token, zero-padded to 3200 rows
    selm = np.zeros((RPAD, TLOC), np.float32)
    rows = np.arange(RHALF)
    selm[rows, rows // SHALF] = 1.0 / S
    selm = np.ascontiguousarray(
        selm.reshape(NT, 128, TLOC).transpose(1, 0, 2).astype(bf))
    identm = np.ascontiguousarray(np.eye(128, dtype=np.float32).astype(bf))

    rel_eff = rel_pos_t / SCALE                            # (127, HD)
    wq_b = np.ascontiguousarray(W_qkv[:, :D].astype(bf))
    wkv_b = np.ascontiguousarray(W_qkv[:, D:].astype(bf))
    wp_b = np.ascontiguousarray(W_proj.astype(bf))
    planes = np.stack([
        np.tile(np.asarray(g_q, np.float32), NH),
        np.tile(np.asarray(b_q, np.float32), NH),
        np.tile(np.asarray(b_k, np.float32), NH),
        np.tile(np.asarray(b_v, np.float32), NH),
        np.tile(np.asarray(g_v, np.float32), NH),
    ], axis=0)                                             # (5, D)
    prow_b = np.ascontiguousarray(
        np.broadcast_to(planes[None], (TLOC, 5, D)).astype(bf))
    bproj_b = np.ascontiguousarray(
        np.broadcast_to(np.asarray(b_proj, np.float32), (TLOC, D)))
    gk_col = np.ascontiguousarray(np.asarray(g_k, np.float32).reshape(HD, 1))

    in_maps = []
    jj = np.arange(128)
    for c in range(N_CORES):
        b = c // 2
        t0 = (c % 2) * TLOC
        # R flipped per core: R_c[d, j] = rel_eff[clip(t0 + 126 - j), d]
        idx = np.clip(t0 + 126 - jj, 0, NDIST - 1)
        rptt_c = np.ascontiguousarray(rel_eff[idx].T.astype(bf))   # (HD, 128)
        xt = x[b, t0:t0 + TLOC].reshape(TLOC, S, D)
        xa_c = np.zeros((RPAD, D), bf)
        xb_c = np.zeros((RPAD, D), bf)
        xa_c[:RHALF] = xt[:, :SHALF].reshape(RHALF, D).astype(bf)
        xb_c[:RHALF] = xt[:, SHALF:].reshape(RHALF, D).astype(bf)
        in_maps.append({
            "xa": np.ascontiguousarray(xa_c),
            "xb": np.ascontiguousarray(xb_c),
            "sel": selm,
            "idp": identm,
            "wq": wq_b,
            "wkv": wkv_b,
            "wp": wp_b,
            "rptt": rptt_c,
            "prow": prow_b,
            "bproj": bproj_b,
            "gkcol": gk_col,
        })
    return in_maps


def _get_nc():
    if "nc" not in _BUILD_CACHE:
        _BUILD_CACHE["nc"] = _build_nc()
    return _BUILD_CACHE["nc"]


def run_on_device(in_maps, **kw):
    from concourse.bass_utils import run_bass_kernel_spmd
    nc = _get_nc()
    return run_bass_kernel_spmd(nc, in_maps, list(range(N_CORES)), **kw)


def kernel(**inputs):
    in_maps = _host_prep(**inputs)
    res = run_on_device(in_maps)
    out = np.zeros((B, T, D), np.float32)
    for c in range(N_CORES):
        b = c // 2
        t0 = (c % 2) * TLOC
        out[b, t0:t0 + TLOC] = res.results[c]["out"]
    return out


# revision 14
# speedup vs baseline: 1.0707x; 1.0707x over previous
"""AdaptivePoolAttention Trainium2 kernel (8 NeuronCores, SPMD). v2

Structure (per core: batch b=c//2, token half c%2 -> 32 of 64 tokens):
 - AdaptiveAvgPool commutes with the qkv linear, so pool x first: the device
   streams x (bf16) and reduces 196 spatial rows -> 1 per token.
 - Spatial pre-add in the DMA engine: host splits each token's 196 rows into
   two halves of 98; half A is DMA'd plain, half B with accum_op=add into the
   same SBUF tiles. Halves the selector-matmul work (25 tiles instead of 49)
   at zero engine cost.
 - Selector matmul (bf16, TensorE) finishes the 98-row group means.
 - Weights/params stream strictly AFTER x (they'd otherwise delay the pooled
   result); ordered by tail need: wq first, wkv next, wp last.
 - Pairwise AllGather (cores 2c, 2c+1) of pooled halves -> full (64, 768).
 - Tail engine split: q path (proj+LN+rel-pos bias) on gpsimd/PE overlapping
   the collective; kv path (proj+LN) on vector/PE; exp/sqrt on ACT with table
   loads sequenced sqrt,sqrt,exp to avoid thrash; bf16 intermediates halve
   DVE cost. The rel-pos bias bias[t,s,h] is one GEMM G = q @ R^T over 127
   distances plus an affine-strided gather DMA through DRAM.
"""

import numpy as np
from contextlib import ExitStack

B, T, NH, HD, D = 4, 64, 12, 64, 768
S = 196            # 14*14 spatial positions
SHALF = 98         # spatial half pre-added by DMA
TLOC = 32          # tokens per core
RHALF = TLOC * SHALF   # 3136 valid rows per half
NT = 25            # 128-row tiles per half (3200 rows padded)
RPAD = NT * 128    # 3200
KB = D // 128      # 6 contraction tiles of 128
NDIST = 2 * T - 1  # 127 distinct temporal distances
SCALE = HD ** -0.5
LN_EPS = 1e-5
N_CORES = 8
GROUPS = (1, 2, 4, 6, 6, 3, 2, 1)  # progressive tile groups, sum = NT
USE_DMA_ACCUM = False

_BUILD_CACHE = {}


def _build_nc():
    import concourse.bass as bass
    import concourse.bacc as bacc
    import concourse.tile as tile
    import concourse.mybir as mybir
    from concourse.tile_rust import add_dep_helper

    f32 = mybir.dt.float32
    bf16 = mybir.dt.bfloat16
    AX = mybir.AxisListType.X
    OP = mybir.AluOpType

    nc = bacc.Bacc(
        "TRN2", target_bir_lowering=False, debug=False, num_devices=N_CORES,
    )

    xa = nc.declare_dram_parameter("xa", [RPAD, D], bf16, isOutput=False)
    xb = nc.declare_dram_parameter("xb", [RPAD, D], bf16, isOutput=False)
    sel = nc.declare_dram_parameter("sel", [128, NT, TLOC], bf16, isOutput=False)
    idp = nc.declare_dram_parameter("idp", [128, 128], bf16, isOutput=False)
    wq = nc.declare_dram_parameter("wq", [D, D], bf16, isOutput=False)
    wkv = nc.declare_dram_parameter("wkv", [D, 2 * D], bf16, isOutput=False)
    wp = nc.declare_dram_parameter("wp", [D, D], bf16, isOutput=False)
    rptt = nc.declare_dram_parameter("rptt", [HD, 128], bf16, isOutput=False)
    # prow planes: 0=gq 1=bq 2=bkrow 3=bvrow 4=gvrow (bf16, row-broadcast)
    prow = nc.declare_dram_parameter("prow", [TLOC, 5, D], bf16, isOutput=False)
    bproj = nc.declare_dram_parameter("bproj", [TLOC, D], f32, isOutput=False)
    gkcol = nc.declare_dram_parameter("gkcol", [HD, 1], f32, isOutput=False)
    out_ext = nc.declare_dram_parameter("out", [TLOC, D], f32, isOutput=True)

    with ExitStack() as ctx:
        tc = ctx.enter_context(tile.TileContext(nc))
        const = ctx.enter_context(tc.tile_pool(name="const", bufs=1))
        xp = ctx.enter_context(tc.tile_pool(name="xp", bufs=1))
        sb = ctx.enter_context(tc.tile_pool(name="sb", bufs=1))
        pg = ctx.enter_context(tc.tile_pool(name="pg", bufs=2, space="PSUM"))
        pt = ctx.enter_context(tc.tile_pool(name="pt", bufs=2, space="PSUM"))
        dram = ctx.enter_context(tc.tile_pool(name="dram", bufs=1, space="DRAM"))

        eps_sb = const.tile([128, 1], f32, tag="eps")
        nc.vector.memset(eps_sb, LN_EPS)
        zero_sb = const.tile([128, 1], f32, tag="zero")
        nc.vector.memset(zero_sb, 0.0)

        # head-of-line DMAs: sel (sync ring), identity (scalar ring)
        sel_sb = const.tile([128, NT, TLOC], bf16, tag="sel")
        nc.sync.dma_start(out=sel_sb, in_=sel.ap())
        ident = const.tile([128, 128], bf16, tag="ident")
        nc.scalar.dma_start(out=ident, in_=idp.ap())

        # preload the ACT Sqrt table (both LNs use it before the exp switch)
        warm = sb.tile([1, 1], f32, tag="warm")
        nc.scalar.activation(
            out=warm, in_=zero_sb[0:1, :],
            func=mybir.ActivationFunctionType.Sqrt,
            bias=eps_sb[0:1, :], scale=1.0,
        )

        # tiny dummy AllGather wakes ncfw so the real collective is fast
        agw_in = dram.tile([1, 32], bf16, tag="agwi")
        agw_out = dram.tile([2, 32], bf16, tag="agwo")
        nc.gpsimd.dma_start(out=agw_in, in_=sel_sb[0:1, 0, :])
        nc.gpsimd.collective_compute(
            "AllGather",
            mybir.AluOpType.bypass,
            replica_groups=[[0, 1], [2, 3], [4, 5], [6, 7]],
            ins=[agw_in.opt()],
            outs=[agw_out.opt()],
        )

        # ---- phase 1: stream x halves; DMA pre-adds half B into half A ----
        xa_t = xa.ap().rearrange("(n p) d -> n p d", p=128)
        xb_t = xb.ap().rearrange("(n p) d -> n p d", p=128)
        m_psum = pg.tile([TLOC, D], f32, tag="g")
        xa_dmas, xb_dmas = [], []
        t_base = 0
        for g, gsz in enumerate(GROUPS):
            xt = xp.tile([128, gsz, D], bf16, tag=f"x{g}")
            src_a = bass.AP(
                tensor=xa_t.tensor,
                offset=xa_t.offset + t_base * 128 * D,
                ap=[[D, 128], [128 * D, gsz], [1, D]],
            )
            src_b = bass.AP(
                tensor=xb_t.tensor,
                offset=xb_t.offset + t_base * 128 * D,
                ap=[[D, 128], [128 * D, gsz], [1, D]],
            )
            ring_a = nc.sync if g % 2 == 0 else nc.scalar
            da = ring_a.dma_start(out=xt[:, 0:gsz, :], in_=src_a)
            if USE_DMA_ACCUM:
                # accum DMA is software-DGE (gpsimd) only
                db = nc.gpsimd.dma_start(
                    out=xt[:, 0:gsz, :], in_=src_b,
                    accum_op=mybir.AluOpType.add,
                )
                add_dep_helper(db.ins, da.ins, sync=True, reason="accum after base")
                xs_g = xt
            else:
                xbt = xp.tile([128, gsz, D], bf16, tag=f"xb{g}")
                ring_b = nc.scalar if g % 2 == 0 else nc.sync
                db = ring_b.dma_start(out=xbt[:, 0:gsz, :], in_=src_b)
                xs_g = xp.tile([128, gsz, D], bf16, tag=f"xs{g}")
            xa_dmas.append(da)
            xb_dmas.append(db)
            for i in range(gsz):
                ti = t_base + i
                if not USE_DMA_ACCUM:
                    # per-tile pre-add so the selector matmul pipelines
                    nc.vector.tensor_tensor(
                        out=xs_g[:, i, :], in0=xt[:, i, :],
                        in1=xbt[:, i, :], op=OP.add,
                    )
                for c0, cw in ((0, 512), (512, 256)):
                    nc.tensor.matmul(
                        m_psum[:, c0:c0 + cw],
                        sel_sb[:, ti, :],
                        xs_g[:, i, c0:c0 + cw],
                        start=(ti == 0),
                        stop=(ti == NT - 1),
                    )
            t_base += gsz

        # ---- weights/params strictly after BOTH rings' x streams,
        # ordered by tail need: wq | small params, wkv | wp last ----
        def after_x(wd):
            add_dep_helper(wd.ins, xa_dmas[-1].ins, sync=False, reason="x before w")
            add_dep_helper(wd.ins, xb_dmas[-1].ins, sync=False, reason="x before w")

        wq_sb = const.tile([128, KB, D], bf16, tag="wq")
        after_x(nc.sync.dma_start(out=wq_sb, in_=wq.ap().rearrange("(k p) e -> p k e", p=128)))
        prow_sb = const.tile([TLOC, 5, D], bf16, tag="prow")
        after_x(nc.scalar.dma_start(out=prow_sb, in_=prow.ap()))
        rptt_sb = const.tile([HD, 128], bf16, tag="rptt")
        after_x(nc.scalar.dma_start(out=rptt_sb, in_=rptt.ap()))
        gkcol_sb = const.tile([HD, 1], f32, tag="gkcol")
        after_x(nc.scalar.dma_start(out=gkcol_sb, in_=gkcol.ap()))
        bproj_sb = const.tile([TLOC, D], f32, tag="bproj")
        after_x(nc.scalar.dma_start(out=bproj_sb, in_=bproj.ap()))
        wkv_sb = const.tile([128, KB, 2 * D], bf16, tag="wkv")
        after_x(nc.scalar.dma_start(out=wkv_sb, in_=wkv.ap().rearrange("(k p) e -> p k e", p=128)))
        wp_sb = const.tile([128, KB, D], bf16, tag="wp")
        after_x(nc.sync.dma_start(out=wp_sb, in_=wp.ap().rearrange("(k p) e -> p k e", p=128)))

        def bcast_free(ap2d, inner):
            # (P, F) AP -> (P, F, inner) AP with stride-0 innermost broadcast
            return bass.AP(
                tensor=ap2d.tensor,
                offset=ap2d.offset,
                ap=[*ap2d.ap, [0, inner]],
            )

        def plane(i):
            return prow_sb[:, i, :]

        def plane3(i):
            return prow_sb[:, i, :].rearrange("p (g d) -> p g d", g=NH)

        # ================= tail =================
        # T0: pooled m (32, 768) complete in PSUM
        m_sb = sb.tile([TLOC, D], bf16, tag="m")
        nc.vector.tensor_copy(out=m_sb, in_=m_psum)

        # collective path (gpsimd ring): m -> DRAM -> AllGather -> mf
        ag_in = dram.tile([TLOC, D], bf16, tag="agi")
        ag_out = dram.tile([T, D], bf16, tag="ago")
        nc.gpsimd.dma_start(out=ag_in, in_=m_sb)
        nc.gpsimd.collective_compute(
            "AllGather",
            mybir.AluOpType.bypass,
            replica_groups=[[0, 1], [2, 3], [4, 5], [6, 7]],
            ins=[ag_in.opt()],
            outs=[ag_out.opt()],
        )
        mf_sb = sb.tile([T, D], bf16, tag="mf")
        nc.gpsimd.dma_start(out=mf_sb, in_=ag_out)

        # local m^T then q projection
        mT_psum = pt.tile([128, KB, TLOC], bf16, tag="t")
        for k in range(KB):
            nc.tensor.matmul(
                mT_psum[:, k, :], m_sb[:, k * 128:(k + 1) * 128],
                ident[:TLOC, :TLOC], is_transpose=True,
            )
        mT_sb = sb.tile([128, KB, TLOC], bf16, tag="mT")
        nc.vector.tensor_copy(out=mT_sb, in_=mT_psum)

        q_psum = pg.tile([TLOC, D], f32, tag="g")
        for k in range(KB):
            for c0, cw in ((0, 512), (512, 256)):
                nc.tensor.matmul(
                    q_psum[:, c0:c0 + cw],
                    mT_sb[:, k, :],
                    wq_sb[:, k, c0:c0 + cw],
                    start=(k == 0), stop=(k == KB - 1),
                )

        # gathered tokens^T (PE waits on the collective here)
        mfT_psum = pt.tile([128, KB, T], bf16, tag="t")
        for k in range(KB):
            nc.tensor.matmul(
                mfT_psum[:, k, :], mf_sb[:, k * 128:(k + 1) * 128],
                ident[:T, :T], is_transpose=True,
            )
        mfT_sb = sb.tile([128, KB, T], bf16, tag="mfT")
        nc.vector.tensor_copy(out=mfT_sb, in_=mfT_psum)

        # kv projection: k rows 0:T, v rows T:128 of one (128, 768) PSUM tile
        kv_psum = pg.tile([128, D], f32, tag="g")
        for k in range(KB):
            for c0, cw in ((0, 512), (512, 256)):
                nc.tensor.matmul(
                    kv_psum[0:T, c0:c0 + cw],
                    mfT_sb[:, k, :],
                    wkv_sb[:, k, c0:c0 + cw],
                    start=(k == 0), stop=(k == KB - 1),
                )
            for c0, cw in ((0, 512), (512, 256)):
                nc.tensor.matmul(
                    kv_psum[T:128, c0:c0 + cw],
                    mfT_sb[:, k, :],
                    wkv_sb[:, k, D + c0:D + c0 + cw],
                    start=(k == 0), stop=(k == KB - 1),
                )

        # ---- q LayerNorm on gpsimd (sqrt on ACT, recip on vector) ----
        q3 = q_psum.rearrange("p (g d) -> p g d", g=NH)
        q_red = sb.tile([TLOC, NH], f32, tag="q_red")
        nc.vector.reduce_sum(out=q_red, in_=q3, axis=AX)
        q_mean = sb.tile([TLOC, NH], f32, tag="q_mean")
        nc.gpsimd.tensor_scalar_mul(out=q_mean, in0=q_red, scalar1=1.0 / HD)
        q_xc = sb.tile([TLOC, NH, HD], bf16, tag="q_xc")
        nc.vector.tensor_tensor(
            out=q_xc, in0=q3, in1=bcast_free(q_mean[:], HD), op=OP.subtract,
        )
        q_sq = sb.tile([TLOC, NH, HD], bf16, tag="q_sq")
        nc.gpsimd.tensor_mul(out=q_sq, in0=q_xc, in1=q_xc)
        q_var = sb.tile([TLOC, NH], f32, tag="q_var")
        nc.vector.reduce_sum(out=q_var, in_=q_sq, axis=AX)
        nc.scalar.activation(
            out=q_var, in_=q_var, func=mybir.ActivationFunctionType.Sqrt,
            bias=eps_sb[:TLOC], scale=1.0 / HD,
        )
        q_rstd = sb.tile([TLOC, NH], bf16, tag="q_rstd")
        with nc.allow_low_precision(reason="bf16 rstd: 0.4% on a norm scale"):
            nc.vector.reciprocal(out=q_rstd, in_=q_var)
        q_xcr = sb.tile([TLOC, NH, HD], bf16, tag="q_xcr")
        nc.gpsimd.tensor_tensor(
            out=q_xcr, in0=q_xc, in1=bcast_free(q_rstd[:], HD), op=OP.mult,
        )
        q_g = sb.tile([TLOC, NH, HD], bf16, tag="q_g")
        nc.gpsimd.tensor_mul(out=q_g, in0=q_xcr, in1=plane3(0))
        ln_q = sb.tile([TLOC, D], bf16, tag="lnq")
        nc.gpsimd.tensor_add(
            out=ln_q.rearrange("p (g d) -> p g d", g=NH), in0=q_g, in1=plane3(1),
        )
        # Bq[t,h] = sum_d ln_q * b_k  (k-LN beta folded into the bias)
        qbk = sb.tile([TLOC, NH, HD], bf16, tag="qbk")
        nc.gpsimd.tensor_mul(
            out=qbk, in0=ln_q.rearrange("p (g d) -> p g d", g=NH), in1=plane3(2),
        )
        bq_fold = sb.tile([TLOC, NH], bf16, tag="bqf")
        with nc.allow_low_precision(reason="bf16 folded beta_k bias term"):
            nc.vector.reduce_sum(out=bq_fold, in_=qbk, axis=AX)
        # residual + beta_v tile
        lnq_bv = sb.tile([TLOC, D], bf16, tag="lnqbv")
        nc.gpsimd.tensor_add(out=lnq_bv, in0=ln_q, in1=plane(3))

        # q^T per head
        qbT_psum = pt.tile([HD, NH, TLOC], bf16, tag="t")
        for h in range(NH):
            nc.tensor.matmul(
                qbT_psum[:, h, :], ln_q[:, h * HD:(h + 1) * HD],
                ident[:TLOC, :TLOC], is_transpose=True,
            )
        qbT_sb = sb.tile([HD, NH, TLOC], bf16, tag="qbT")
        nc.vector.tensor_copy(out=qbT_sb, in_=qbT_psum)

        # rel-pos bias: G = q @ Rflip^T then affine gather through DRAM
        g_psum = pg.tile([TLOC, NH, 128], f32, tag="g")
        for h in range(NH):
            nc.tensor.matmul(
                g_psum[:, h, :], qbT_sb[:, h, :], rptt_sb,
                start=True, stop=True,
            )
        g_sb = sb.tile([TLOC, NH, 128], bf16, tag="gsb")
        nc.scalar.activation(
            out=g_sb.rearrange("p h j -> p (h j)"),
            in_=g_psum.rearrange("p h j -> p (h j)"),
            func=mybir.ActivationFunctionType.Copy,
            bias=0.0, scale=1.0,
        )
        g_dram = dram.tile([TLOC, NH, 128], bf16, tag="gd")
        nc.gpsimd.dma_start(out=g_dram, in_=g_sb)
        bias_raw = sb.tile([TLOC, NH, T], bf16, tag="braw")
        gather_ap = bass.AP(
            tensor=g_dram.tensor,
            offset=g_dram.offset + 63,
            ap=[[NH * 128 - 1, TLOC], [128, NH], [1, T]],
        )
        nc.gpsimd.dma_start(out=bias_raw, in_=gather_ap)
        bias_sb = sb.tile([TLOC, NH, T], f32, tag="bias")
        nc.gpsimd.tensor_tensor(
            out=bias_sb, in0=bias_raw, in1=bcast_free(bq_fold[:], T), op=OP.add,
        )

        # ---- kv LayerNorm (normalized only) on vector ----
        kv3 = kv_psum.rearrange("p (g d) -> p g d", g=NH)
        kv_red = sb.tile([128, NH], f32, tag="kv_red")
        nc.vector.reduce_sum(out=kv_red, in_=kv3, axis=AX)
        kv_mean = sb.tile([128, NH], f32, tag="kv_mean")
        nc.vector.tensor_scalar_mul(out=kv_mean, in0=kv_red, scalar1=1.0 / HD)
        kv_xc = sb.tile([128, NH, HD], bf16, tag="kv_xc")
        nc.vector.tensor_tensor(
            out=kv_xc, in0=kv3, in1=bcast_free(kv_mean[:], HD), op=OP.subtract,
        )
        kv_sq = sb.tile([128, NH, HD], bf16, tag="kv_sq")
        nc.vector.tensor_mul(out=kv_sq, in0=kv_xc, in1=kv_xc)
        kv_var = sb.tile([128, NH], f32, tag="kv_var")
        nc.vector.reduce_sum(out=kv_var, in_=kv_sq, axis=AX)
        nc.scalar.activation(
            out=kv_var, in_=kv_var, func=mybir.ActivationFunctionType.Sqrt,
            bias=eps_sb, scale=1.0 / HD,
        )
        # trigger the exp table load now (softmax comes right after)
        expwarm = sb.tile([1, 1], f32, tag="expwarm")
        nc.scalar.activation(
            out=expwarm, in_=zero_sb[0:1, :],
            func=mybir.ActivationFunctionType.Exp,
            bias=zero_sb[0:1, :], scale=1.0,
        )
        kv_rstd = sb.tile([128, NH], bf16, tag="kv_rstd")
        with nc.allow_low_precision(reason="bf16 rstd: 0.4% on a norm scale"):
            nc.vector.reciprocal(out=kv_rstd, in_=kv_var)
        ln_kv = sb.tile([128, D], bf16, tag="lnkv")
        nc.vector.tensor_tensor(
            out=ln_kv.rearrange("p (g d) -> p g d", g=NH),
            in0=kv_xc, in1=bcast_free(kv_rstd[:], HD), op=OP.mult,
        )
        ln_v = sb.tile([T, D], bf16, tag="lnv")
        nc.vector.tensor_copy(out=ln_v, in_=ln_kv[T:128, :])

        # k^T per head with gamma_k applied per-partition
        kT_psum = pt.tile([HD, NH, T], bf16, tag="t")
        for h in range(NH):
            nc.tensor.matmul(
                kT_psum[:, h, :], ln_kv[0:T, h * HD:(h + 1) * HD],
                ident[:T, :T], is_transpose=True,
            )
        kT_sb = sb.tile([HD, NH, T], bf16, tag="kT")
        nc.vector.tensor_scalar_mul(
            out=kT_sb.rearrange("p h s -> p (h s)"),
            in0=kT_psum.rearrange("p h s -> p (h s)"),
            scalar1=gkcol_sb,
        )

        # scores, bias add, exp (scale folded into the activation)
        s_psum = pg.tile([TLOC, NH, T], f32, tag="g")
        for h in range(NH):
            nc.tensor.matmul(
                s_psum[:, h, :], qbT_sb[:, h, :], kT_sb[:, h, :],
                start=True, stop=True,
            )
        s_sb = sb.tile([TLOC, NH, T], bf16, tag="ssb")
        nc.vector.tensor_tensor(out=s_sb, in0=s_psum, in1=bias_sb, op=OP.add)
        p_sb = sb.tile([TLOC, NH, T], bf16, tag="p")
        nc.scalar.activation(
            out=p_sb.rearrange("p h s -> p (h s)"),
            in_=s_sb.rearrange("p h s -> p (h s)"),
            func=mybir.ActivationFunctionType.Exp,
            bias=zero_sb[:TLOC], scale=SCALE,
        )
        rsum = sb.tile([TLOC, NH], f32, tag="rsum")
        nc.vector.reduce_sum(out=rsum, in_=p_sb, axis=AX)
        rinv = sb.tile([TLOC, NH], f32, tag="rinv")
        nc.vector.reciprocal(out=rinv, in_=rsum)

        # P^T per head
        pT_psum = pt.tile([T, NH, TLOC], bf16, tag="t")
        for h in range(NH):
            nc.tensor.matmul(
                pT_psum[:, h, :], p_sb[:, h, :],
                ident[:TLOC, :TLOC], is_transpose=True,
            )
        pT_sb = sb.tile([T, NH, TLOC], bf16, tag="pT")
        nc.vector.tensor_copy(out=pT_sb, in_=pT_psum)

        # A@V per head
        o_psum = pg.tile([TLOC, NH, HD], f32, tag="g")
        for h in range(NH):
            nc.tensor.matmul(
                o_psum[:, h, :], pT_sb[:, h, :],
                ln_v[:, h * HD:(h + 1) * HD],
                start=True, stop=True,
            )
        # o = (P@v_hat)*(1/sum)*gamma_v + (ln_q + beta_v)
        o1 = sb.tile([TLOC, NH, HD], bf16, tag="o1")
        nc.vector.tensor_tensor(
            out=o1, in0=o_psum, in1=bcast_free(rinv[:], HD), op=OP.mult,
        )
        o2 = sb.tile([TLOC, NH, HD], bf16, tag="o2")
        nc.gpsimd.tensor_mul(out=o2, in0=o1, in1=plane3(4))
        o_sb = sb.tile([TLOC, D], bf16, tag="o")
        nc.gpsimd.tensor_add(
            out=o_sb, in0=o2.rearrange("p h d -> p (h d)"), in1=lnq_bv,
        )

        # o^T then output projection
        oT_psum = pt.tile([128, KB, TLOC], bf16, tag="t")
        for k in range(KB):
            nc.tensor.matmul(
                oT_psum[:, k, :], o_sb[:, k * 128:(k + 1) * 128],
                ident[:TLOC, :TLOC], is_transpose=True,
            )
        oT_sb = sb.tile([128, KB, TLOC], bf16, tag="oT")
        nc.vector.tensor_copy(out=oT_sb, in_=oT_psum)

        proj_psum = pg.tile([TLOC, D], f32, tag="g")
        out_sb = sb.tile([TLOC, D], f32, tag="outsb")
        for c0, cw in ((0, 512), (512, 256)):
            for k in range(KB):
                nc.tensor.matmul(
                    proj_psum[:, c0:c0 + cw],
                    oT_sb[:, k, :],
                    wp_sb[:, k, c0:c0 + cw],
                    start=(k == 0), stop=(k == KB - 1),
                )
            nc.vector.tensor_add(
                out=out_sb[:, c0:c0 + cw], in0=proj_psum[:, c0:c0 + cw],
                in1=bproj_sb[:, c0:c0 + cw],
            )
            nc.sync.dma_start(
                out=out_ext.ap()[:, c0:c0 + cw], in_=out_sb[:, c0:c0 + cw],
            )

    nc.compile()
    return nc


def _host_prep(x, W_qkv, g_q, b_q, g_k, b_k, g_v, b_v, W_proj, b_proj, rel_pos_t):
    import ml_dtypes
    bf = ml_dtypes.bfloat16
    x = np.asarray(x, np.float32)
    W_qkv = np.asarray(W_qkv, np.float32)
    W_proj = np.asarray(W_proj, np.float32)
    rel_pos_t = np.asarray(rel_pos_t, np.float32)

    # selector: 98 pre-added rows per token, zero-padded to 3200 rows
    selm = np.zeros((RPAD, TLOC), np.float32)
    rows = np.arange(RHALF)
    selm[rows, rows // SHALF] = 1.0 / S
    selm = np.ascontiguousarray(
        selm.reshape(NT, 128, TLOC).transpose(1, 0, 2).astype(bf))
    identm = np.ascontiguousarray(np.eye(128, dtype=np.float32).astype(bf))

    rel_eff = rel_pos_t / SCALE                            # (127, HD)
    wq_b = np.ascontiguousarray(W_qkv[:, :D].astype(bf))
    wkv_b = np.ascontiguousarray(W_qkv[:, D:].astype(bf))
    wp_b = np.ascontiguousarray(W_proj.astype(bf))
    planes = np.stack([
        np.tile(np.asarray(g_q, np.float32), NH),
        np.tile(np.asarray(b_q, np.float32), NH),
        np.tile(np.asarray(b_k, np.float32), NH),
        np.tile(np.asarray(b_v, np.float32), NH),
        np.tile(np.asarray(g_v, np.float32), NH),
    ], axis=0)                                             # (5, D)
    prow_b = np.ascontiguousarray(
        np.broadcast_to(planes[None], (TLOC, 5, D)).astype(bf))
    bproj_b = np.ascontiguousarray(
        np.broadcast_to(np.asarray(b_proj, np.float32), (TLOC, D)))
    gk_col = np.ascontiguousarray(np.asarray(g_k, np.float32).reshape(HD, 1))

    in_maps = []
    jj = np.arange(128)
    for c in range(N_CORES):
        b = c // 2
        t0 = (c % 2) * TLOC
        # R flipped per core: R_c[d, j] = rel_eff[clip(t0 + 126 - j), d]
        idx = np.clip(t0 + 126 - jj, 0, NDIST - 1)
        rptt_c = np.ascontiguousarray(rel_eff[idx].T.astype(bf))   # (HD, 128)
        xt = x[b, t0:t0 + TLOC].reshape(TLOC, S, D)
        xa_c = np.zeros((RPAD, D), bf)
        xb_c = np.zeros((RPAD, D), bf)
        xa_c[:RHALF] = xt[:, :SHALF].reshape(RHALF, D).astype(bf)
        xb_c[:RHALF] = xt[:, SHALF:].reshape(RHALF, D).astype(bf)
        in_maps.append({
            "xa": np.ascontiguousarray(xa_c),
            "xb": np.ascontiguousarray(xb_c),
            "sel": selm,
            "idp": identm,
            "wq": wq_b,
            "wkv": wkv_b,
            "wp": wp_b,
            "rptt": rptt_c,
            "prow": prow_b,
            "bproj": bproj_b,
            "gkcol": gk_col,
        })
    return in_maps


def _get_nc():
    if "nc" not in _BUILD_CACHE:
        _BUILD_CACHE["nc"] = _build_nc()
    return _BUILD_CACHE["nc"]


def run_on_device(in_maps, **kw):
    from concourse.bass_utils import run_bass_kernel_spmd
    nc = _get_nc()
    return run_bass_kernel_spmd(nc, in_maps, list(range(N_CORES)), **kw)


def kernel(**inputs):
    in_maps = _host_prep(**inputs)
    res = run_on_device(in_maps)
    out = np.zeros((B, T, D), np.float32)
    for c in range(N_CORES):
        b = c // 2
        t0 = (c % 2) * TLOC
        out[b, t0:t0 + TLOC] = res.results[c]["out"]
    return out


# revision 17
# speedup vs baseline: 1.2877x; 1.2027x over previous
"""AdaptivePoolAttention Trainium2 kernel (8 NeuronCores, SPMD). v3

Per core: batch b=c//2, token half c%2 (32 of 64 tokens).
 - Pooling commutes with the qkv linear: pool x first. Host pre-splits each
   token's 196 spatial rows into two 98-row halves laid out partition-major
   (one contiguous DMA descriptor per partition); DVE adds the halves, a
   bf16 selector matmul on TensorE finishes the 98-row group means (25 tiles).
 - Weights stream strictly after x, ordered by tail need (wq, wkv | wp).
 - Pairwise AllGather (cores 2c,2c+1) of pooled halves; gpsimd queue carries
   only the collective chain so nothing blocks behind it.
 - Tail: q LN on vector with gamma_q/beta_q applied per-partition during the
   q^T PSUM->SBUF copy (scalar_tensor_tensor), beta_k folded into column 127
   of the rel-pos matrix so the whole k-side bias rides the G GEMM + one
   strided-gather DMA through DRAM. kv LN normalized-only; gamma_k per-
   partition on the k^T copy; gamma_v/beta_v folded into the output path.
   PSUM->SBUF copies that ACT can do ride the scalar engine; sqrt,sqrt,exp
   table order avoids ACT table thrash.
"""

import numpy as np
from contextlib import ExitStack

B, T, NH, HD, D = 4, 64, 12, 64, 768
S = 196            # 14*14 spatial positions
SHALF = 98         # spatial half per pre-add
TLOC = 32          # tokens per core
RHALF = TLOC * SHALF   # 3136 valid rows per half
NT = 25            # 128-row tiles per half (3200 rows padded)
RPAD = NT * 128    # 3200
KB = D // 128      # 6 contraction tiles of 128
NDIST = 2 * T - 1  # 127 distinct temporal distances
SCALE = HD ** -0.5
LN_EPS = 1e-5
N_CORES = 8
GROUPS = (6, 6, 5, 5, 2, 1)  # tile groups, sum = NT; one DMA start per group

_BUILD_CACHE = {}


def _build_nc():
    import concourse.bass as bass
    import concourse.bacc as bacc
    import concourse.tile as tile
    import concourse.mybir as mybir
    from concourse.tile_rust import add_dep_helper

    f32 = mybir.dt.float32
    bf16 = mybir.dt.bfloat16
    AX = mybir.AxisListType.X
    OP = mybir.AluOpType
    ACTF = mybir.ActivationFunctionType

    nc = bacc.Bacc(
        "TRN2", target_bir_lowering=False, debug=False, num_devices=N_CORES,
    )

    # xab: partition-major (128, 2*NT, 768); per group g the columns
    # [2*base : 2*base+2*gsz] hold [xa tiles | xb tiles] of that group.
    xab = nc.declare_dram_parameter("xab", [128, 2 * NT, D], bf16, isOutput=False)
    sel = nc.declare_dram_parameter("sel", [128, NT, TLOC], bf16, isOutput=False)
    idp = nc.declare_dram_parameter("idp", [128, 128], bf16, isOutput=False)
    # weights partition-major: [p][k][e]
    wq = nc.declare_dram_parameter("wq", [128, KB, D], bf16, isOutput=False)
    wkv = nc.declare_dram_parameter("wkv", [128, KB, 2 * D], bf16, isOutput=False)
    wp = nc.declare_dram_parameter("wp", [128, KB, D], bf16, isOutput=False)
    rptt = nc.declare_dram_parameter("rptt", [HD, 128], bf16, isOutput=False)
    # prow planes: 0=gq_row 1=bqv_row(beta_q+beta_v) 2=gv_row (bf16)
    prow = nc.declare_dram_parameter("prow", [TLOC, 3, D], bf16, isOutput=False)
    bproj = nc.declare_dram_parameter("bproj", [TLOC, D], f32, isOutput=False)
    gkcol = nc.declare_dram_parameter("gkcol", [HD, 1], f32, isOutput=False)
    gqcol = nc.declare_dram_parameter("gqcol", [HD, 1], f32, isOutput=False)
    bqcol = nc.declare_dram_parameter("bqcol", [HD, 1], bf16, isOutput=False)
    out_ext = nc.declare_dram_parameter("out", [TLOC, D], f32, isOutput=True)

    with ExitStack() as ctx:
        tc = ctx.enter_context(tile.TileContext(nc))
        const = ctx.enter_context(tc.tile_pool(name="const", bufs=1))
        xp = ctx.enter_context(tc.tile_pool(name="xp", bufs=1))
        sb = ctx.enter_context(tc.tile_pool(name="sb", bufs=1))
        pg = ctx.enter_context(tc.tile_pool(name="pg", bufs=2, space="PSUM"))
        pt = ctx.enter_context(tc.tile_pool(name="pt", bufs=2, space="PSUM"))
        dram = ctx.enter_context(tc.tile_pool(name="dram", bufs=1, space="DRAM"))

        eps_sb = const.tile([128, 1], f32, tag="eps")
        nc.vector.memset(eps_sb, LN_EPS)
        zero_sb = const.tile([128, 1], f32, tag="zero")
        nc.vector.memset(zero_sb, 0.0)

        sel_sb = const.tile([128, NT, TLOC], bf16, tag="sel")
        nc.sync.dma_start(out=sel_sb, in_=sel.ap())
        ident = const.tile([128, 128], bf16, tag="ident")
        nc.scalar.dma_start(out=ident, in_=idp.ap())

        # preload the ACT Sqrt table
        warm = sb.tile([1, 1], f32, tag="warm")
        nc.scalar.activation(
            out=warm, in_=zero_sb[0:1, :], func=ACTF.Sqrt,
            bias=eps_sb[0:1, :], scale=1.0,
        )

        # dummy AllGather wakes ncfw early so the real collective is fast
        agw_src = const.tile([1, 32], bf16, tag="agws")
        nc.vector.memset(agw_src, 0.0)
        agw_in = dram.tile([1, 32], bf16, tag="agwi")
        agw_out = dram.tile([2, 32], bf16, tag="agwo")
        nc.gpsimd.dma_start(out=agw_in, in_=agw_src)
        nc.gpsimd.collective_compute(
            "AllGather",
            mybir.AluOpType.bypass,
            replica_groups=[[0, 1], [2, 3], [4, 5], [6, 7]],
            ins=[agw_in.opt()],
            outs=[agw_out.opt()],
        )

        # ---- phase 1: x stream (one start per group) + DVE pre-add +
        # selector matmul accumulation ----
        m_psum = pg.tile([TLOC, D], f32, tag="g")
        x_dmas = []
        t_base = 0
        for g, gsz in enumerate(GROUPS):
            xt = xp.tile([128, 2 * gsz, D], bf16, tag=f"x{g}")
            src = bass.AP(
                tensor=xab.ap().tensor,
                offset=xab.ap().offset + 2 * t_base * D,
                ap=[[2 * NT * D, 128], [D, 2 * gsz], [1, D]],
            )
            ring = nc.sync if g % 2 == 0 else nc.scalar
            x_dmas.append(ring.dma_start(out=xt, in_=src))
            xs_g = xp.tile([128, gsz, D], bf16, tag=f"xs{g}")
            for i in range(gsz):
                ti = t_base + i
                nc.vector.tensor_tensor(
                    out=xs_g[:, i, :], in0=xt[:, i, :],
                    in1=xt[:, gsz + i, :], op=OP.add,
                )
                for c0, cw in ((0, 512), (512, 256)):
                    nc.tensor.matmul(
                        m_psum[:, c0:c0 + cw],
                        sel_sb[:, ti, :],
                        xs_g[:, i, c0:c0 + cw],
                        start=(ti == 0),
                        stop=(ti == NT - 1),
                    )
            t_base += gsz

        def after_x(wd):
            for dd in x_dmas[-2:]:
                add_dep_helper(wd.ins, dd.ins, sync=False, reason="x before w")

        # sync ring after x: wq, mf-read (emitted later), params, wp
        wq_sb = const.tile([128, KB, D], bf16, tag="wq")
        after_x(nc.sync.dma_start(out=wq_sb, in_=wq.ap()))

        # ================= tail =================
        m_sb = sb.tile([TLOC, D], bf16, tag="m")
        nc.scalar.activation(   # ACT copies PSUM->SBUF so vector stays free
            out=m_sb, in_=m_psum, func=ACTF.Copy, bias=0.0, scale=1.0,
        )
        # wkv on the scalar ring, emitted after m-copy so its descriptor
        # generation does not delay the copy
        wkv_sb = const.tile([128, KB, 2 * D], bf16, tag="wkv")
        after_x(nc.scalar.dma_start(out=wkv_sb, in_=wkv.ap()))

        # collective chain on gpsimd only
        ag_in = dram.tile([TLOC, D], bf16, tag="agi")
        ag_out = dram.tile([T, D], bf16, tag="ago")
        nc.gpsimd.dma_start(out=ag_in, in_=m_sb)
        nc.gpsimd.collective_compute(
            "AllGather",
            mybir.AluOpType.bypass,
            replica_groups=[[0, 1], [2, 3], [4, 5], [6, 7]],
            ins=[ag_in.opt()],
            outs=[ag_out.opt()],
        )

        # local m^T then q projection
        mT_psum = pt.tile([128, KB, TLOC], bf16, tag="t")
        for k in range(KB):
            nc.tensor.matmul(
                mT_psum[:, k, :], m_sb[:, k * 128:(k + 1) * 128],
                ident[:TLOC, :TLOC], is_transpose=True,
            )
        mT_sb = sb.tile([128, KB, TLOC], bf16, tag="mT")
        nc.vector.tensor_copy(out=mT_sb, in_=mT_psum)

        q_psum = pg.tile([TLOC, D], f32, tag="g")
        for k in range(KB):
            for c0, cw in ((0, 512), (512, 256)):
                nc.tensor.matmul(
                    q_psum[:, c0:c0 + cw],
                    mT_sb[:, k, :],
                    wq_sb[:, k, c0:c0 + cw],
                    start=(k == 0), stop=(k == KB - 1),
                )

        # mf read on sync (after wq in queue; waits only the AG semaphore)
        mf_sb = sb.tile([T, D], bf16, tag="mf")
        nc.sync.dma_start(out=mf_sb, in_=ag_out)
        # remaining sync-ring params
        prow_sb = const.tile([TLOC, 3, D], bf16, tag="prow")
        after_x(nc.sync.dma_start(out=prow_sb, in_=prow.ap()))
        rptt_sb = const.tile([HD, 128], bf16, tag="rptt")
        after_x(nc.sync.dma_start(out=rptt_sb, in_=rptt.ap()))
        gkcol_sb = const.tile([HD, 1], f32, tag="gkcol")
        after_x(nc.sync.dma_start(out=gkcol_sb, in_=gkcol.ap()))
        gqcol_sb = const.tile([HD, 1], f32, tag="gqcol")
        after_x(nc.sync.dma_start(out=gqcol_sb, in_=gqcol.ap()))
        bqcol_sb = const.tile([HD, 1], bf16, tag="bqcol")
        after_x(nc.sync.dma_start(out=bqcol_sb, in_=bqcol.ap()))
        bproj_sb = const.tile([TLOC, D], f32, tag="bproj")
        after_x(nc.sync.dma_start(out=bproj_sb, in_=bproj.ap()))
        wp_sb = const.tile([128, KB, D], bf16, tag="wp")
        after_x(nc.sync.dma_start(out=wp_sb, in_=wp.ap()))

        mfT_psum = pt.tile([128, KB, T], bf16, tag="t")
        for k in range(KB):
            nc.tensor.matmul(
                mfT_psum[:, k, :], mf_sb[:, k * 128:(k + 1) * 128],
                ident[:T, :T], is_transpose=True,
            )
        mfT_sb = sb.tile([128, KB, T], bf16, tag="mfT")
        nc.vector.tensor_copy(out=mfT_sb, in_=mfT_psum)

        # kv projection: k rows 0:T, v rows T:128 of one (128, 768) PSUM tile
        kv_psum = pg.tile([128, D], f32, tag="g")
        for k in range(KB):
            for c0, cw in ((0, 512), (512, 256)):
                nc.tensor.matmul(
                    kv_psum[0:T, c0:c0 + cw],
                    mfT_sb[:, k, :],
                    wkv_sb[:, k, c0:c0 + cw],
                    start=(k == 0), stop=(k == KB - 1),
                )
            for c0, cw in ((0, 512), (512, 256)):
                nc.tensor.matmul(
                    kv_psum[T:128, c0:c0 + cw],
                    mfT_sb[:, k, :],
                    wkv_sb[:, k, D + c0:D + c0 + cw],
                    start=(k == 0), stop=(k == KB - 1),
                )

        def bcast_free(ap2d, inner):
            return bass.AP(
                tensor=ap2d.tensor,
                offset=ap2d.offset,
                ap=[*ap2d.ap, [0, inner]],
            )

        # ---- q LayerNorm (normalized part) on vector ----
        q3 = q_psum.rearrange("p (g d) -> p g d", g=NH)
        q_red = sb.tile([TLOC, NH], f32, tag="q_red")
        nc.vector.reduce_sum(out=q_red, in_=q3, axis=AX)
        q_mean = sb.tile([TLOC, NH], f32, tag="q_mean")
        nc.vector.tensor_scalar_mul(out=q_mean, in0=q_red, scalar1=1.0 / HD)
        q_xc = sb.tile([TLOC, NH, HD], bf16, tag="q_xc")
        nc.vector.tensor_tensor(
            out=q_xc, in0=q3, in1=bcast_free(q_mean[:], HD), op=OP.subtract,
        )
        q_sq = sb.tile([TLOC, NH, HD], bf16, tag="q_sq")
        nc.vector.tensor_mul(out=q_sq, in0=q_xc, in1=q_xc)
        q_var = sb.tile([TLOC, NH], f32, tag="q_var")
        nc.vector.reduce_sum(out=q_var, in_=q_sq, axis=AX)
        nc.scalar.activation(
            out=q_var, in_=q_var, func=ACTF.Sqrt,
            bias=eps_sb[:TLOC], scale=1.0 / HD,
        )
        q_rstd = sb.tile([TLOC, NH], bf16, tag="q_rstd")
        with nc.allow_low_precision(reason="bf16 rstd: 0.4% on a norm scale"):
            nc.vector.reciprocal(out=q_rstd, in_=q_var)
        q_hat = sb.tile([TLOC, NH, HD], bf16, tag="q_hat")
        nc.vector.tensor_tensor(
            out=q_hat, in0=q_xc, in1=bcast_free(q_rstd[:], HD), op=OP.mult,
        )

        # q^T per head; gamma_q,beta_q applied per-partition in the copy
        qbT_psum = pt.tile([HD, NH, TLOC], bf16, tag="t")
        qh2 = q_hat.rearrange("p g d -> p (g d)")
        for h in range(NH):
            nc.tensor.matmul(
                qbT_psum[:, h, :], qh2[:, h * HD:(h + 1) * HD],
                ident[:TLOC, :TLOC], is_transpose=True,
            )
        qbT_sb = sb.tile([HD, NH, TLOC], bf16, tag="qbT")
        bq_b = bass.AP(
            tensor=bqcol_sb[:].tensor, offset=bqcol_sb[:].offset,
            ap=[bqcol_sb[:].ap[0], [0, NH], [0, TLOC]],
        )
        nc.vector.scalar_tensor_tensor(
            out=qbT_sb, in0=qbT_psum, scalar=gqcol_sb[:],
            in1=bq_b, op0=OP.mult, op1=OP.add,
        )

        # G[t,h,j] = sum_d ln_q^T * Rflip (col 127 of Rflip = beta_k, giving
        # the folded k-beta bias term in G[:, :, 127])
        g_psum = pg.tile([TLOC, NH, 128], f32, tag="g")
        for h in range(NH):
            nc.tensor.matmul(
                g_psum[:, h, :], qbT_sb[:, h, :], rptt_sb,
                start=True, stop=True,
            )
        g_sb = sb.tile([TLOC, NH, 128], bf16, tag="gsb")
        nc.scalar.activation(
            out=g_sb.rearrange("p h j -> p (h j)"),
            in_=g_psum.rearrange("p h j -> p (h j)"),
            func=ACTF.Copy, bias=0.0, scale=1.0,
        )
        g_dram = dram.tile([TLOC, NH, 128], bf16, tag="gd")
        nc.gpsimd.dma_start(out=g_dram, in_=g_sb)
        bias_raw = sb.tile([TLOC, NH, T], bf16, tag="braw")
        gather_ap = bass.AP(
            tensor=g_dram.tensor,
            offset=g_dram.offset + 63,
            ap=[[NH * 128 - 1, TLOC], [128, NH], [1, T]],
        )
        nc.gpsimd.dma_start(out=bias_raw, in_=gather_ap)

        # ---- kv LayerNorm (normalized only) on vector ----
        kv3 = kv_psum.rearrange("p (g d) -> p g d", g=NH)
        kv_red = sb.tile([128, NH], f32, tag="kv_red")
        nc.vector.reduce_sum(out=kv_red, in_=kv3, axis=AX)
        kv_mean = sb.tile([128, NH], f32, tag="kv_mean")
        nc.vector.tensor_scalar_mul(out=kv_mean, in0=kv_red, scalar1=1.0 / HD)
        kv_xc = sb.tile([128, NH, HD], bf16, tag="kv_xc")
        nc.vector.tensor_tensor(
            out=kv_xc, in0=kv3, in1=bcast_free(kv_mean[:], HD), op=OP.subtract,
        )
        kv_sq = sb.tile([128, NH, HD], bf16, tag="kv_sq")
        nc.vector.tensor_mul(out=kv_sq, in0=kv_xc, in1=kv_xc)
        kv_var = sb.tile([128, NH], f32, tag="kv_var")
        nc.vector.reduce_sum(out=kv_var, in_=kv_sq, axis=AX)
        nc.scalar.activation(
            out=kv_var, in_=kv_var, func=ACTF.Sqrt,
            bias=eps_sb, scale=1.0 / HD,
        )
        # trigger the exp table load now (softmax comes soon)
        expwarm = sb.tile([1, 1], f32, tag="expwarm")
        nc.scalar.activation(
            out=expwarm, in_=zero_sb[0:1, :], func=ACTF.Exp,
            bias=0.0, scale=1.0,
        )
        kv_rstd = sb.tile([128, NH], bf16, tag="kv_rstd")
        with nc.allow_low_precision(reason="bf16 rstd: 0.4% on a norm scale"):
            nc.vector.reciprocal(out=kv_rstd, in_=kv_var)
        ln_kv = sb.tile([128, D], bf16, tag="lnkv")
        nc.vector.tensor_tensor(
            out=ln_kv.rearrange("p (g d) -> p g d", g=NH),
            in0=kv_xc, in1=bcast_free(kv_rstd[:], HD), op=OP.mult,
        )
        ln_v = sb.tile([T, D], bf16, tag="lnv")
        nc.vector.tensor_copy(out=ln_v, in_=ln_kv[T:128, :])

        # k^T per head with gamma_k applied per-partition
        kT_psum = pt.tile([HD, NH, T], bf16, tag="t")
        for h in range(NH):
            nc.tensor.matmul(
                kT_psum[:, h, :], ln_kv[0:T, h * HD:(h + 1) * HD],
                ident[:T, :T], is_transpose=True,
            )
        kT_sb = sb.tile([HD, NH, T], bf16, tag="kT")
        nc.vector.tensor_scalar_mul(
            out=kT_sb.rearrange("p h s -> p (h s)"),
            in0=kT_psum.rearrange("p h s -> p (h s)"),
            scalar1=gkcol_sb,
        )

        # scores + (gathered rel-pos bias + folded beta_k bias) then exp
        s_psum = pg.tile([TLOC, NH, T], f32, tag="g")
        for h in range(NH):
            nc.tensor.matmul(
                s_psum[:, h, :], qbT_sb[:, h, :], kT_sb[:, h, :],
                start=True, stop=True,
            )
        g127 = bass.AP(
            tensor=g_sb[:].tensor, offset=g_sb[:].offset + 127,
            ap=[g_sb[:].ap[0], [128, NH], [0, T]],
        )
        bias_sb = sb.tile([TLOC, NH, T], bf16, tag="bias")
        nc.vector.tensor_tensor(
            out=bias_sb, in0=bias_raw, in1=g127, op=OP.add,
        )
        s_sb = sb.tile([TLOC, NH, T], bf16, tag="ssb")
        nc.vector.tensor_tensor(out=s_sb, in0=s_psum, in1=bias_sb, op=OP.add)
        p_sb = sb.tile([TLOC, NH, T], bf16, tag="p")
        nc.scalar.activation(
            out=p_sb.rearrange("p h s -> p (h s)"),
            in_=s_sb.rearrange("p h s -> p (h s)"),
            func=ACTF.Exp,
            bias=zero_sb[:TLOC], scale=SCALE,
        )
        # residual tile (fills the exp wait): q_hat*gamma_q + (beta_q+beta_v)
        resid = sb.tile([TLOC, D], bf16, tag="resid")
        nc.vector.tensor_mul(
            out=resid, in0=q_hat.rearrange("p g d -> p (g d)"),
            in1=prow_sb[:, 0, :],
        )
        nc.vector.tensor_add(out=resid, in0=resid, in1=prow_sb[:, 1, :])
        rsum = sb.tile([TLOC, NH], f32, tag="rsum")
        nc.vector.reduce_sum(out=rsum, in_=p_sb, axis=AX)
        rinv = sb.tile([TLOC, NH], f32, tag="rinv")
        nc.vector.reciprocal(out=rinv, in_=rsum)

        # P^T per head
        pT_psum = pt.tile([T, NH, TLOC], bf16, tag="t")
        for h in range(NH):
            nc.tensor.matmul(
                pT_psum[:, h, :], p_sb[:, h, :],
                ident[:TLOC, :TLOC], is_transpose=True,
            )
        pT_sb = sb.tile([T, NH, TLOC], bf16, tag="pT")
        nc.vector.tensor_copy(out=pT_sb, in_=pT_psum)

        # A@V per head
        o_psum = pg.tile([TLOC, NH, HD], f32, tag="g")
        for h in range(NH):
            nc.tensor.matmul(
                o_psum[:, h, :], pT_sb[:, h, :],
                ln_v[:, h * HD:(h + 1) * HD],
                start=True, stop=True,
            )
        o1 = sb.tile([TLOC, NH, HD], bf16, tag="o1")
        nc.vector.tensor_tensor(
            out=o1, in0=o_psum, in1=bcast_free(rinv[:], HD), op=OP.mult,
        )
        o2 = sb.tile([TLOC, NH, HD], bf16, tag="o2")
        nc.vector.tensor_mul(
            out=o2, in0=o1,
            in1=prow_sb[:, 2, :].rearrange("p (g d) -> p g d", g=NH),
        )
        o_sb = sb.tile([TLOC, D], bf16, tag="o")
        nc.vector.tensor_add(
            out=o_sb, in0=o2.rearrange("p h d -> p (h d)"), in1=resid,
        )

        # o^T then output projection
        oT_psum = pt.tile([128, KB, TLOC], bf16, tag="t")
        for k in range(KB):
            nc.tensor.matmul(
                oT_psum[:, k, :], o_sb[:, k * 128:(k + 1) * 128],
                ident[:TLOC, :TLOC], is_transpose=True,
            )
        oT_sb = sb.tile([128, KB, TLOC], bf16, tag="oT")
        nc.vector.tensor_copy(out=oT_sb, in_=oT_psum)

        proj_psum = pg.tile([TLOC, D], f32, tag="g")
        out_sb = sb.tile([TLOC, D], f32, tag="outsb")
        for c0, cw in ((0, 512), (512, 256)):
            for k in range(KB):
                nc.tensor.matmul(
                    proj_psum[:, c0:c0 + cw],
                    oT_sb[:, k, :],
                    wp_sb[:, k, c0:c0 + cw],
                    start=(k == 0), stop=(k == KB - 1),
                )
            nc.vector.tensor_add(
                out=out_sb[:, c0:c0 + cw], in0=proj_psum[:, c0:c0 + cw],
                in1=bproj_sb[:, c0:c0 + cw],
            )
            nc.sync.dma_start(
                out=out_ext.ap()[:, c0:c0 + cw], in_=out_sb[:, c0:c0 + cw],
            )

    nc.compile()
    return nc


def _host_prep(x, W_qkv, g_q, b_q, g_k, b_k, g_v, b_v, W_proj, b_proj, rel_pos_t):
    import ml_dtypes
    bf = ml_dtypes.bfloat16
    x = np.asarray(x, np.float32)
    W_qkv = np.asarray(W_qkv, np.float32)
    W_proj = np.asarray(W_proj, np.float32)
    rel_pos_t = np.asarray(rel_pos_t, np.float32)
    g_q = np.asarray(g_q, np.float32)
    b_q = np.asarray(b_q, np.float32)
    g_k = np.asarray(g_k, np.float32)
    b_k = np.asarray(b_k, np.float32)
    g_v = np.asarray(g_v, np.float32)
    b_v = np.asarray(b_v, np.float32)
    b_proj = np.asarray(b_proj, np.float32)

    selm = np.zeros((RPAD, TLOC), np.float32)
    rows = np.arange(RHALF)
    selm[rows, rows // SHALF] = 1.0 / S
    selm = np.ascontiguousarray(
        selm.reshape(NT, 128, TLOC).transpose(1, 0, 2).astype(bf))
    identm = np.ascontiguousarray(np.eye(128, dtype=np.float32).astype(bf))

    def pmajor(w, cols):
        # (768, cols) row-major -> (128, KB, cols) partition-major
        return np.ascontiguousarray(
            w.reshape(KB, 128, cols).transpose(1, 0, 2).astype(bf))

    wq_b = pmajor(W_qkv[:, :D], D)
    wkv_b = pmajor(W_qkv[:, D:], 2 * D)
    wp_b = pmajor(W_proj, D)

    planes = np.stack([
        np.tile(g_q, NH),
        np.tile(b_q + b_v, NH),
        np.tile(g_v, NH),
    ], axis=0)                                             # (3, D)
    prow_b = np.ascontiguousarray(
        np.broadcast_to(planes[None], (TLOC, 3, D)).astype(bf))
    bproj_b = np.ascontiguousarray(np.broadcast_to(b_proj, (TLOC, D)))
    gk_col = np.ascontiguousarray(g_k.reshape(HD, 1))
    gq_col = np.ascontiguousarray(g_q.reshape(HD, 1))
    bq_col = np.ascontiguousarray(b_q.reshape(HD, 1).astype(bf))

    rel_eff = rel_pos_t / SCALE                            # (127, HD)

    in_maps = []
    jj = np.arange(128)
    for c in range(N_CORES):
        b = c // 2
        t0 = (c % 2) * TLOC
        # R flipped per core: R_c[d, j] = rel_eff[clip(t0 + 126 - j), d];
        # column 127 (never addressed by the gather) carries beta_k so the
        # G GEMM also produces the folded q.beta_k bias term.
        idx = np.clip(t0 + 126 - jj, 0, NDIST - 1)
        rptt_c = rel_eff[idx].T.copy()                     # (HD, 128)
        rptt_c[:, 127] = b_k
        rptt_c = np.ascontiguousarray(rptt_c.astype(bf))

        xt = x[b, t0:t0 + TLOC].reshape(TLOC, S, D)
        xa_c = np.zeros((RPAD, D), np.float32)
        xb_c = np.zeros((RPAD, D), np.float32)
        xa_c[:RHALF] = xt[:, :SHALF].reshape(RHALF, D)
        xb_c[:RHALF] = xt[:, SHALF:].reshape(RHALF, D)
        xa_t = xa_c.reshape(NT, 128, D)
        xb_t = xb_c.reshape(NT, 128, D)
        # per-group [xa tiles | xb tiles], partition-major
        xab_c = np.empty((128, 2 * NT, D), np.float32)
        base = 0
        for gsz in GROUPS:
            xab_c[:, 2 * base:2 * base + gsz] = (
                xa_t[base:base + gsz].transpose(1, 0, 2))
            xab_c[:, 2 * base + gsz:2 * base + 2 * gsz] = (
                xb_t[base:base + gsz].transpose(1, 0, 2))
            base += gsz
        in_maps.append({
            "xab": np.ascontiguousarray(xab_c.astype(bf)),
            "sel": selm,
            "idp": identm,
            "wq": wq_b,
            "wkv": wkv_b,
            "wp": wp_b,
            "rptt": rptt_c,
            "prow": prow_b,
            "bproj": bproj_b,
            "gkcol": gk_col,
            "gqcol": gq_col,
            "bqcol": bq_col,
        })
    return in_maps


def _get_nc():
    if "nc" not in _BUILD_CACHE:
        _BUILD_CACHE["nc"] = _build_nc()
    return _BUILD_CACHE["nc"]


def run_on_device(in_maps, **kw):
    from concourse.bass_utils import run_bass_kernel_spmd
    nc = _get_nc()
    return run_bass_kernel_spmd(nc, in_maps, list(range(N_CORES)), **kw)


def kernel(**inputs):
    in_maps = _host_prep(**inputs)
    res = run_on_device(in_maps)
    out = np.zeros((B, T, D), np.float32)
    for c in range(N_CORES):
        b = c // 2
        t0 = (c % 2) * TLOC
        out[b, t0:t0 + TLOC] = res.results[c]["out"]
    return out
